# revision 1
# baseline (speedup 1.0000x reference)
"""Trainium2 Bass kernel for LocalSelfAttention (sliding-window attention).

Reference computation (fp32):
  qkv = x @ W_qkv ; q /= 8 ; sliding window of 7 keys (3 each side, zero-padded)
  attn = softmax(q . k_win + pos_bias) ; out = (attn @ v_win) @ W_out

Sharding: data-parallel over B*HW = 128 independent rows -> 16 rows per core.
Each core processes its rows in 8 pairs (512 tokens per pair).

Per-core layout strategy (bf16 matmuls, fp32 PSUM accumulation):
  - x [tokens, D] -> cast bf16 -> PE-transpose -> xT [D, tokens]
  - qkT = W_qk^T. @ xT  (q,k dims on partitions, tokens free)
  - V   = xT^T. @ W_v   (tokens on partitions, v dims free)
  - scores ST[key, q] = kT_h^T. @ qT_h per (head, key-chunk, row) on query stripes
  - attn_un = exp(ST) * expB   (expB = host-precomputed exp(pos_bias) band mask;
                                zero outside the 7-band -> masks everything)
  - denom = ones[128,64]^T. @ attn_un (replicated across 64 partitions by the
    matmul) + host edge-correction for zero-padded window slots; reciprocal
  - avT[dk, q] = V_chunk^T. @ attn_un; normalized by recip during PSUM->SBUF
  - out = avT^T. @ W_out
"""

import numpy as np
import ml_dtypes

import concourse.bass as bass
import concourse.tile as tile
from concourse import bacc, mybir
from concourse.bass_utils import run_bass_kernel_spmd
from concourse.masks import make_identity

# Problem constants (hardcoded per contract)
B, HW, S, D = 2, 64, 256, 512
HEADS, DK, KSIZE, PAD = 8, 64, 7, 3
HDK = HEADS * DK            # 512
QK = 2 * HDK                # 1024 (q and k dims)
N_CORES = 8
ROWS_PER_CORE = (B * HW) // N_CORES   # 16
PAIRS = ROWS_PER_CORE // 2            # 8
PTOK = 2 * S                          # 512 tokens per pair
P = 128
NCH = S // P                          # 2 key chunks per row
STRIPE = 132                          # query stripe width per key chunk (even)
STRIPE_PAD = 256                      # psum slot per (chunk,row) stripe, bank aligned
STRIPE_START = (0, S - STRIPE)        # stripe start per chunk within a row

F32 = mybir.dt.float32
BF16 = mybir.dt.bfloat16

_CACHE = {}


def _host_constants(pos_bias, W_qkv, W_out):
    """Host-precomputed tensors: bf16 weights (q pre-scaled), expB band mask,
    replicated edge correction for zero-padded window slots."""
    W1 = W_qkv.astype(np.float32).copy()
    W1[:, :HDK] /= np.sqrt(np.float32(DK))
    W1 = W1.astype(ml_dtypes.bfloat16)            # [512, 1536]
    W2 = W_out.astype(np.float32).astype(ml_dtypes.bfloat16)  # [512, 512]

    pb = pos_bias.astype(np.float32)              # [H, S, KSIZE]
    # expB[j, h, c, q'] : key j (within chunk c), query q = STRIPE_START[c] + q'
    # value exp(pos_bias[h, q, w]) with w = (j_global - q) + PAD if in band else 0
    j = np.arange(P)[:, None, None, None]
    h = np.arange(HEADS)[None, :, None, None]
    c = np.arange(NCH)[None, None, :, None]
    qp = np.arange(STRIPE)[None, None, None, :]
    q_glob = np.array(STRIPE_START)[None, None, :, None] + qp
    j_glob = c * P + j
    w = j_glob - q_glob + PAD
    in_band = (w >= 0) & (w < KSIZE)
    w_c = np.clip(w, 0, KSIZE - 1)
    bias_val = pb[h, q_glob, w_c]
    expB = np.where(in_band, np.exp(bias_val), 0.0).astype(np.float32)
    expB = expB.astype(ml_dtypes.bfloat16)        # [128, H, NCH, STRIPE]

    # edge correction: sum over out-of-range window slots of exp(bias),
    # replicated across all 128 partitions for partition-aligned adds
    q = np.arange(S)[None, :, None]
    w2 = np.arange(KSIZE)[None, None, :]
    oor = ((q + w2 - PAD) < 0) | ((q + w2 - PAD) >= S)
    ec = (np.exp(pb) * oor).sum(-1)               # [H, S]
    ec_pair = np.concatenate([ec, ec], axis=1)    # [H, PTOK]
    ec_rep = np.broadcast_to(ec_pair[None], (P, HEADS, PTOK))
    return W1, W2, expB, np.ascontiguousarray(ec_rep, dtype=np.float32)


def _build_nc():
    nc = bacc.Bacc(None, target_bir_lowering=False)
    x_d = nc.dram_tensor("x", [ROWS_PER_CORE * S, D], F32, kind="ExternalInput")
    w1_d = nc.dram_tensor("w1", [D, 3 * HDK], BF16, kind="ExternalInput")
    w2_d = nc.dram_tensor("w2", [HDK, D], BF16, kind="ExternalInput")
    expb_d = nc.dram_tensor("expb", [P, HEADS, NCH, STRIPE], BF16, kind="ExternalInput")
    ec_d = nc.dram_tensor("ec", [P, HEADS, PTOK], F32, kind="ExternalInput")
    out_d = nc.dram_tensor("out", [ROWS_PER_CORE * S, D], F32, kind="ExternalOutput")

    KO = D // P      # 4 K-chunks for projections
    TC = PTOK // P   # 4 token chunks per pair
    QKC = QK // P    # 8 qk output chunks
    HC = HDK // P    # 4 hdk chunks

    with tile.TileContext(nc) as tc:
        with (
            tc.tile_pool(name="const", bufs=1) as const,
            tc.tile_pool(name="io", bufs=2) as io,
            tc.tile_pool(name="work", bufs=2) as work,
            tc.tile_pool(name="attn", bufs=2) as attnp,
            tc.tile_pool(name="ps_proj", bufs=3, space="PSUM") as ps_proj,
            tc.tile_pool(name="ps_st", bufs=2, space="PSUM") as ps_st,
            tc.tile_pool(name="dramp", bufs=2, space="DRAM") as dramp,
        ):
            # ---- constants ----
            w1_sb = const.tile([P, KO, 3 * HDK], BF16)
            nc.sync.dma_start(w1_sb[:], w1_d.rearrange("(ko ki) n -> ki ko n", ki=P))
            w2_sb = const.tile([P, HC, D], BF16)
            nc.sync.dma_start(w2_sb[:], w2_d.rearrange("(hc ki) n -> ki hc n", ki=P))
            expb_sb = const.tile([P, HEADS, NCH, STRIPE], BF16)
            nc.sync.dma_start(expb_sb[:], expb_d[:])
            ec_sb = const.tile([P, HEADS, PTOK], F32)
            nc.sync.dma_start(ec_sb[:], ec_d[:])
            ident = const.tile([P, P], BF16)
            make_identity(nc, ident)
            ones_sb = const.tile([P, 64], BF16)
            nc.vector.memset(ones_sb, 1.0)

            for pr in range(PAIRS):
                # ---- load x pair, cast to bf16 ----
                x_f32 = io.tile([P, TC, D], F32, tag="x_f32")
                nc.sync.dma_start(
                    x_f32[:],
                    x_d[pr * PTOK:(pr + 1) * PTOK, :].rearrange(
                        "(tc p) d -> p tc d", p=P),
                )
                x_bf = work.tile([P, TC, D], BF16, tag="x_bf")
                nc.any.tensor_copy(x_bf[:], x_f32[:])

                # ---- transpose x -> xT [D partitions, tokens] ----
                xT = work.tile([P, KO, PTOK], BF16, tag="xT")
                for tcc in range(TC):
                    tp = ps_proj.tile([P, KO, P], BF16, tag="p512")
                    for ds in range(KO):
                        nc.tensor.transpose(
                            tp[:, ds, :], x_bf[:, tcc, ds * P:(ds + 1) * P], ident)
                    nc.any.tensor_copy(xT[:, :, tcc * P:(tcc + 1) * P], tp[:])

                # ---- qk projection: qkT [qk dims, tokens] ----
                qkT = work.tile([P, QKC, PTOK], BF16, tag="qkT")
                for m in range(QKC):
                    pp = ps_proj.tile([P, PTOK], F32, tag="p512")
                    for ko in range(KO):
                        nc.tensor.matmul(
                            pp[:],
                            w1_sb[:, ko, m * P:(m + 1) * P],
                            xT[:, ko, :],
                            start=(ko == 0), stop=(ko == KO - 1),
                        )
                    nc.any.tensor_copy(qkT[:, m, :], pp[:])

                # ---- v projection: V [tokens, hdk] ----
                v_sb = work.tile([P, TC, HDK], BF16, tag="v_sb")
                for tcc in range(TC):
                    pp = ps_proj.tile([P, PTOK], F32, tag="p512")
                    for ko in range(KO):
                        nc.tensor.matmul(
                            pp[:],
                            xT[:, ko, tcc * P:(tcc + 1) * P],
                            w1_sb[:, ko, QK:],
                            start=(ko == 0), stop=(ko == KO - 1),
                        )
                    nc.any.tensor_copy(v_sb[:, tcc, :], pp[:])

                # ---- attention: 2 halves of 4 heads ----
                avT = attnp.tile([P, HC, PTOK], BF16, tag="avT")
                for half in range(2):
                    attn_uns = []
                    for k in range(4):
                        h = half * 4 + k
                        mq = h // 2          # q chunk index in qkT
                        mk = 4 + h // 2      # k chunk index in qkT
                        p0 = 64 * (h % 2)    # partition offset within chunk
                        sl = slice(p0, p0 + 64)

                        st = ps_st.tile([P, NCH, 2, STRIPE_PAD], F32, tag="st")
                        for c in range(NCH):
                            for r in range(2):
                                nc.tensor.matmul(
                                    st[:, c, r, :STRIPE],
                                    qkT[sl, mk,
                                        r * S + c * P:r * S + (c + 1) * P],
                                    qkT[sl, mq,
                                        r * S + STRIPE_START[c]:
                                        r * S + STRIPE_START[c] + STRIPE],
                                    start=True, stop=True,
                                )
                        # exp(ST) -> bf16, then multiply by expB band mask
                        attn_un = attnp.tile(
                            [P, NCH, 2, STRIPE], BF16, tag=f"attn_un{k % 2}")
                        nc.scalar.activation(
                            attn_un[:], st[:, :, :, :STRIPE],
                            func=mybir.ActivationFunctionType.Exp)
                        nc.vector.tensor_tensor(
                            attn_un[:], attn_un[:],
                            expb_sb[:, h, :, None, :].to_broadcast(
                                (P, NCH, 2, STRIPE)),
                            mybir.AluOpType.mult,
                        )
                        attn_uns.append(attn_un)
                    # denominators per head-pair, replicated on 64 partitions
                    # by an M=64 ones matmul; one ec-add + recip per head
                    recip_rep = attnp.tile([P, 2, PTOK], F32, tag="recip_rep")
                    for k in range(4):
                        h = half * 4 + k
                        p0 = 64 * (h % 2)
                        sl = slice(p0, p0 + 64)
                        tpos = None if p0 == 0 else (0, 64)
                        if k % 2 == 0:
                            den = ps_proj.tile([P, PTOK], F32, tag="p512")
                        first = True
                        for r in range(2):
                            for c in range(NCH):
                                nc.tensor.matmul(
                                    den[sl, r * S + STRIPE_START[c]:
                                            r * S + STRIPE_START[c] + STRIPE],
                                    ones_sb[:],
                                    attn_uns[k][:, c, r, :],
                                    start=first,
                                    stop=(r == 1 and c == NCH - 1),
                                    tile_position=tpos,
                                )
                                first = False
                        nc.vector.tensor_tensor(
                            recip_rep[sl, k // 2, :], den[sl, :],
                            ec_sb[sl, half * 4 + k, :], mybir.AluOpType.add)
                        nc.vector.reciprocal(
                            recip_rep[sl, k // 2, :], recip_rep[sl, k // 2, :])
                    for k in range(4):
                        h = half * 4 + k
                        p0 = 64 * (h % 2)
                        sl = slice(p0, p0 + 64)
                        tpos = None if p0 == 0 else (0, 64)
                        attn_un = attn_uns[k]
                        # avT[dk, tokens] = sum over key chunks V^T. @ attn_un
                        if h % 2 == 0:
                            avp = ps_proj.tile([P, PTOK], F32, tag="p512")
                        first = True
                        for r in range(2):
                            for c in range(NCH):
                                nc.tensor.matmul(
                                    avp[sl, r * S + STRIPE_START[c]:
                                            r * S + STRIPE_START[c] + STRIPE],
                                    v_sb[:, 2 * r + c, h * DK:(h + 1) * DK],
                                    attn_un[:, c, r, :],
                                    start=first,
                                    stop=(r == 1 and c == NCH - 1),
                                    tile_position=tpos,
                                )
                                first = False
                        # normalize while copying PSUM -> SBUF
                        nc.vector.tensor_tensor(
                            avT[sl, h // 2, :], avp[sl, :],
                            recip_rep[sl, k // 2, :],
                            mybir.AluOpType.mult,
                        )

                # ---- output projection ----
                o_sb = io.tile([P, TC, D], F32, tag="o_sb")
                for tcc in range(TC):
                    pp = ps_proj.tile([P, PTOK], F32, tag="p512")
                    for hc in range(HC):
                        nc.tensor.matmul(
                            pp[:],
                            avT[:, hc, tcc * P:(tcc + 1) * P],
                            w2_sb[:, hc, :],
                            start=(hc == 0), stop=(hc == HC - 1),
                        )
                    nc.any.tensor_copy(o_sb[:, tcc, :], pp[:])
                nc.sync.dma_start(
                    out_d[pr * PTOK:(pr + 1) * PTOK, :].rearrange(
                        "(tc p) d -> p tc d", p=P),
                    o_sb[:],
                )

    nc.compile()
    return nc


def kernel(inputs, pos_bias, W_qkv, W_out):
    x = np.asarray(inputs, np.float32)
    W1, W2, expB, ec = _host_constants(
        np.asarray(pos_bias), np.asarray(W_qkv), np.asarray(W_out))

    if "nc" not in _CACHE:
        _CACHE["nc"] = _build_nc()
    nc = _CACHE["nc"]

    x_flat = x.reshape(B * HW, S, D)
    in_maps = []
    for core in range(N_CORES):
        shard = x_flat[core * ROWS_PER_CORE:(core + 1) * ROWS_PER_CORE]
        in_maps.append({
            "x": np.ascontiguousarray(shard.reshape(ROWS_PER_CORE * S, D)),
            "w1": W1, "w2": W2, "expb": expB, "ec": ec,
        })
    res = run_bass_kernel_spmd(nc, in_maps, core_ids=list(range(N_CORES)))
    out = np.empty((B * HW, S, D), np.float32)
    for core in range(N_CORES):
        out[core * ROWS_PER_CORE:(core + 1) * ROWS_PER_CORE] = (
            res.results[core]["out"].reshape(ROWS_PER_CORE, S, D))
    return out.reshape(B, HW, S, D)



# revision 7
# speedup vs baseline: 1.8069x; 1.8069x over previous
"""Trainium2 Bass kernel for LocalSelfAttention (sliding-window attention).

Reference computation (fp32):
  qkv = x @ W_qkv ; q /= 8 ; sliding window of 7 keys (3 each side, zero-padded)
  attn = softmax(q . k_win + pos_bias) ; out = (attn @ v_win) @ W_out

Sharding: data-parallel over B*HW = 128 independent rows -> 16 rows per core.
Each core processes its rows in 8 pairs (512 tokens per pair).

Per-core layout (bf16 matmuls, fp32 PSUM accumulation), software-pipelined
4 pairs deep so the PE never stalls on the vector-engine softmax chain:
  stage A(p):  x -> bf16 -> PE-transpose -> xT; qkT = W_qk^T. @ xT;
               V = xT^T. @ W_v; scores ST[key,q] per head; exp (Scalar);
               * expB band mask (Vector)
  stage B1(p): denom = ones^T. @ attn_un (replicated across 64 partitions),
               + host edge-correction, reciprocal_approx_fast (Vector)
  stage B2(p): avT = V^T. @ attn_un; * recip while copying PSUM->SBUF
  stage C(p):  out = avT^T. @ W_out -> DMA

Emission order per step t: A(t), B1(t-1), B2(t-2), C(t-3) -- each
cross-engine dependency gets a full step (~20us) of slack.
"""

import numpy as np
import ml_dtypes

import concourse.bass as bass
import concourse.tile as tile
from concourse import bacc, mybir
from concourse.bass_utils import run_bass_kernel_spmd
from concourse.masks import make_identity

# Problem constants (hardcoded per contract)
B, HW, S, D = 2, 64, 256, 512
HEADS, DK, KSIZE, PAD = 8, 64, 7, 3
HDK = HEADS * DK            # 512
QK = 2 * HDK                # 1024 (q and k dims)
N_CORES = 8
ROWS_PER_CORE = (B * HW) // N_CORES   # 16
PAIRS = ROWS_PER_CORE // 2            # 8
PTOK = 2 * S                          # 512 tokens per pair
P = 128
NCH = S // P                          # 2 key chunks per row
STRIPE = 132                          # query stripe width per key chunk (even)
STRIPE_PAD = 256                      # psum slot per (chunk,row) stripe, bank aligned
STRIPE_START = (0, S - STRIPE)        # stripe start per chunk within a row
HPAIRS = HEADS // 2                   # 4 head pairs packed on 64+64 partitions

F32 = mybir.dt.float32
BF16 = mybir.dt.bfloat16

_CACHE = {}


def _host_constants(pos_bias, W_qkv, W_out):
    """Host-precomputed tensors: bf16 weights (q pre-scaled), expB band mask,
    head-pair-packed edge correction for zero-padded window slots."""
    W1 = W_qkv.astype(np.float32).copy()
    W1[:, :HDK] /= np.sqrt(np.float32(DK))
    W1 = W1.astype(ml_dtypes.bfloat16)            # [512, 1536]
    W2 = W_out.astype(np.float32).astype(ml_dtypes.bfloat16)  # [512, 512]

    pb = pos_bias.astype(np.float32)              # [H, S, KSIZE]
    # expB[j, h, c, q'] : key j (within chunk c), query q = STRIPE_START[c] + q'
    # value exp(pos_bias[h, q, w]) with w = (j_global - q) + PAD if in band else 0
    j = np.arange(P)[:, None, None, None]
    h = np.arange(HEADS)[None, :, None, None]
    c = np.arange(NCH)[None, None, :, None]
    qp = np.arange(STRIPE)[None, None, None, :]
    q_glob = np.array(STRIPE_START)[None, None, :, None] + qp
    j_glob = c * P + j
    w = j_glob - q_glob + PAD
    in_band = (w >= 0) & (w < KSIZE)
    w_c = np.clip(w, 0, KSIZE - 1)
    bias_val = pb[h, q_glob, w_c]
    expB = np.where(in_band, np.exp(bias_val), 0.0).astype(np.float32)
    expB = expB.astype(ml_dtypes.bfloat16)        # [128, H, NCH, STRIPE]

    # edge correction: sum over out-of-range window slots of exp(bias).
    # Packed per head pair: partitions 0:64 <- head 2j, 64:128 <- head 2j+1,
    # matching the tile_position packing of the denominator matmuls.
    q = np.arange(S)[None, :, None]
    w2 = np.arange(KSIZE)[None, None, :]
    oor = ((q + w2 - PAD) < 0) | ((q + w2 - PAD) >= S)
    ec = (np.exp(pb) * oor).sum(-1)               # [H, S]
    ec_pair = np.concatenate([ec, ec], axis=1)    # [H, PTOK]
    ec_pack = np.empty((P, HPAIRS, PTOK), np.float32)
    for jj in range(HPAIRS):
        ec_pack[:64, jj, :] = ec_pair[2 * jj][None, :]
        ec_pack[64:, jj, :] = ec_pair[2 * jj + 1][None, :]
    return W1, W2, expB, ec_pack


def _build_nc():
    nc = bacc.Bacc(None, target_bir_lowering=False)
    x_d = nc.dram_tensor("x", [ROWS_PER_CORE * S, D], F32, kind="ExternalInput")
    w1_d = nc.dram_tensor("w1", [D, 3 * HDK], BF16, kind="ExternalInput")
    w2_d = nc.dram_tensor("w2", [HDK, D], BF16, kind="ExternalInput")
    expb_d = nc.dram_tensor("expb", [P, HEADS, NCH, STRIPE], BF16, kind="ExternalInput")
    ec_d = nc.dram_tensor("ec", [P, HPAIRS, PTOK], F32, kind="ExternalInput")
    out_d = nc.dram_tensor("out", [ROWS_PER_CORE * S, D], F32, kind="ExternalOutput")

    KO = D // P      # 4 K-chunks for projections
    TC = PTOK // P   # 4 token chunks per pair
    QKC = QK // P    # 8 qk output chunks
    HC = HDK // P    # 4 hdk chunks

    with tile.TileContext(nc) as tc:
        with (
            tc.tile_pool(name="const", bufs=1) as const,
            tc.tile_pool(name="io", bufs=3) as io,
            tc.tile_pool(name="early", bufs=2) as early,
            tc.tile_pool(name="vpool", bufs=3) as vpool,
            tc.tile_pool(name="attn", bufs=3) as attnp,
            tc.tile_pool(name="bpool", bufs=2) as bpool,
            tc.tile_pool(name="opool", bufs=2) as opool,
            tc.tile_pool(name="ps_proj", bufs=3, space="PSUM") as ps_proj,
            tc.tile_pool(name="ps_st", bufs=2, space="PSUM") as ps_st,
        ):
            # ---- constants ----
            w1_sb = const.tile([P, KO, 3 * HDK], BF16)
            nc.sync.dma_start(w1_sb[:], w1_d.rearrange("(ko ki) n -> ki ko n", ki=P))
            w2_sb = const.tile([P, HC, D], BF16)
            nc.sync.dma_start(w2_sb[:], w2_d.rearrange("(hc ki) n -> ki hc n", ki=P))
            expb_sb = const.tile([P, HEADS, NCH, STRIPE], BF16)
            nc.sync.dma_start(expb_sb[:], expb_d[:])
            ec_sb = const.tile([P, HPAIRS, PTOK], F32)
            nc.sync.dma_start(ec_sb[:], ec_d[:])
            ident = const.tile([P, P], BF16)
            make_identity(nc, ident)
            ones_sb = const.tile([P, 64], BF16)
            nc.vector.memset(ones_sb, 1.0)

            x_tiles = {}
            attn_tiles = {}
            recip_tiles = {}
            v_tiles = {}
            avT_tiles = {}

            def stage_load(pr):
                x_f32 = io.tile([P, TC, D], F32, tag="x_f32")
                nc.sync.dma_start(
                    x_f32[:],
                    x_d[pr * PTOK:(pr + 1) * PTOK, :].rearrange(
                        "(tc p) d -> p tc d", p=P),
                )
                x_tiles[pr] = x_f32

            def stage_a(pr):
                x_f32 = x_tiles.pop(pr)
                x_bf = early.tile([P, TC, D], BF16, tag="x_bf")
                nc.gpsimd.tensor_copy(x_bf[:], x_f32[:])

                # transpose x -> xT [D partitions, tokens]
                xT = early.tile([P, KO, PTOK], BF16, tag="xT")
                for tcc in range(TC):
                    tp = ps_proj.tile([P, KO, P], BF16, tag="p512")
                    for ds in range(KO):
                        nc.tensor.transpose(
                            tp[:, ds, :], x_bf[:, tcc, ds * P:(ds + 1) * P], ident)
                    nc.scalar.activation(
                        xT[:, :, tcc * P:(tcc + 1) * P], tp[:],
                        func=mybir.ActivationFunctionType.Copy)

                # qk projection: qkT [qk dims, tokens]
                qkT = early.tile([P, QKC, PTOK], BF16, tag="qkT")
                for m in range(QKC):
                    pp = ps_proj.tile([P, PTOK], F32, tag="p512")
                    for ko in range(KO):
                        nc.tensor.matmul(
                            pp[:],
                            w1_sb[:, ko, m * P:(m + 1) * P],
                            xT[:, ko, :],
                            start=(ko == 0), stop=(ko == KO - 1),
                        )
                    nc.scalar.activation(
                        qkT[:, m, :], pp[:],
                        func=mybir.ActivationFunctionType.Copy)

                # v projection: V [tokens, hdk]
                v_sb = vpool.tile([P, TC, HDK], BF16, tag="v_sb")
                v_tiles[pr] = v_sb
                for tcc in range(TC):
                    pp = ps_proj.tile([P, PTOK], F32, tag="p512")
                    for ko in range(KO):
                        nc.tensor.matmul(
                            pp[:],
                            xT[:, ko, tcc * P:(tcc + 1) * P],
                            w1_sb[:, ko, QK:],
                            start=(ko == 0), stop=(ko == KO - 1),
                        )
                    nc.scalar.activation(
                        v_sb[:, tcc, :], pp[:],
                        func=mybir.ActivationFunctionType.Copy)

                # scores + exp + band mask, all 8 heads
                attn_uns = []
                for h in range(HEADS):
                    mq = h // 2          # q chunk index in qkT
                    mk = 4 + h // 2      # k chunk index in qkT
                    p0 = 64 * (h % 2)    # partition offset within chunk
                    sl = slice(p0, p0 + 64)

                    st = ps_st.tile([P, NCH, 2, STRIPE_PAD], F32, tag="st")
                    for c in range(NCH):
                        for r in range(2):
                            nc.tensor.matmul(
                                st[:, c, r, :STRIPE],
                                qkT[sl, mk,
                                    r * S + c * P:r * S + (c + 1) * P],
                                qkT[sl, mq,
                                    r * S + STRIPE_START[c]:
                                    r * S + STRIPE_START[c] + STRIPE],
                                start=True, stop=True,
                            )
                    attn_un = attnp.tile(
                        [P, NCH, 2, STRIPE], BF16, tag=f"attn_un{h}")
                    nc.scalar.activation(
                        attn_un[:], st[:, :, :, :STRIPE],
                        func=mybir.ActivationFunctionType.Exp)
                    nc.gpsimd.tensor_tensor(
                        attn_un[:], attn_un[:],
                        expb_sb[:, h, :, None, :].to_broadcast(
                            (P, NCH, 2, STRIPE)),
                        mybir.AluOpType.mult,
                    )
                    attn_uns.append(attn_un)
                attn_tiles[pr] = attn_uns

            def stage_b1(pr):
                # denominators, packed 2 heads per psum tile (64+64 partitions)
                attn_uns = attn_tiles[pr]
                recip_rep = bpool.tile([P, HPAIRS, PTOK], F32, tag="recip_rep")
                recip_tiles[pr] = recip_rep
                for j in range(HPAIRS):
                    den = ps_proj.tile([P, PTOK], F32, tag="p512")
                    for h in (2 * j, 2 * j + 1):
                        p0 = 64 * (h % 2)
                        sl = slice(p0, p0 + 64)
                        tpos = None if p0 == 0 else (0, 64)
                        first = True
                        for r in range(2):
                            for c in range(NCH):
                                nc.tensor.matmul(
                                    den[sl, r * S + STRIPE_START[c]:
                                            r * S + STRIPE_START[c] + STRIPE],
                                    ones_sb[:],
                                    attn_uns[h][:, c, r, :],
                                    start=first,
                                    stop=(r == 1 and c == NCH - 1),
                                    tile_position=tpos,
                                )
                                first = False
                    nc.vector.tensor_tensor(
                        recip_rep[:, j, :], den[:],
                        ec_sb[:, j, :], mybir.AluOpType.add)
                    nc.vector.reciprocal_approx_fast(
                        recip_rep[:, j, :], recip_rep[:, j, :])

            def stage_b2(pr):
                # avT[dk, tokens] = sum over key chunks V^T. @ attn_un,
                # normalized by recip during PSUM -> SBUF
                attn_uns = attn_tiles.pop(pr)
                recip_rep = recip_tiles.pop(pr)
                v_sb = v_tiles.pop(pr)
                avT = attnp.tile([P, HC, PTOK], BF16, tag="avT")
                avT_tiles[pr] = avT
                for j in range(HPAIRS):
                    avp = ps_proj.tile([P, PTOK], F32, tag="p512")
                    for h in (2 * j, 2 * j + 1):
                        p0 = 64 * (h % 2)
                        sl = slice(p0, p0 + 64)
                        tpos = None if p0 == 0 else (0, 64)
                        first = True
                        for r in range(2):
                            for c in range(NCH):
                                nc.tensor.matmul(
                                    avp[sl, r * S + STRIPE_START[c]:
                                            r * S + STRIPE_START[c] + STRIPE],
                                    v_sb[:, 2 * r + c, h * DK:(h + 1) * DK],
                                    attn_uns[h][:, c, r, :],
                                    start=first,
                                    stop=(r == 1 and c == NCH - 1),
                                    tile_position=tpos,
                                )
                                first = False
                    nc.vector.tensor_tensor(
                        avT[:, j, :], avp[:],
                        recip_rep[:, j, :],
                        mybir.AluOpType.mult,
                    )

            def stage_c(pr):
                avT = avT_tiles.pop(pr)
                o_sb = opool.tile([P, TC, D], F32, tag="o_sb")
                for tcc in range(TC):
                    pp = ps_proj.tile([P, PTOK], F32, tag="p512")
                    for hc in range(HC):
                        nc.tensor.matmul(
                            pp[:],
                            avT[:, hc, tcc * P:(tcc + 1) * P],
                            w2_sb[:, hc, :],
                            start=(hc == 0), stop=(hc == HC - 1),
                        )
                    nc.vector.tensor_copy(o_sb[:, tcc, :], pp[:])
                nc.sync.dma_start(
                    out_d[pr * PTOK:(pr + 1) * PTOK, :].rearrange(
                        "(tc p) d -> p tc d", p=P),
                    o_sb[:],
                )

            # ---- software pipeline: A(t), B1(t-1), B2(t-2), C(t-3) ----
            stage_load(0)
            if PAIRS > 1:
                stage_load(1)
            for t in range(PAIRS + 3):
                if t < PAIRS:
                    stage_a(t)
                    if t + 2 < PAIRS:
                        stage_load(t + 2)
                if 0 <= t - 1 < PAIRS:
                    stage_b1(t - 1)
                if 0 <= t - 2 < PAIRS:
                    stage_b2(t - 2)
                if 0 <= t - 3 < PAIRS:
                    stage_c(t - 3)

    nc.compile()
    return nc


def kernel(inputs, pos_bias, W_qkv, W_out):
    x = np.asarray(inputs, np.float32)
    W1, W2, expB, ec = _host_constants(
        np.asarray(pos_bias), np.asarray(W_qkv), np.asarray(W_out))

    if "nc" not in _CACHE:
        _CACHE["nc"] = _build_nc()
    nc = _CACHE["nc"]

    x_flat = x.reshape(B * HW, S, D)
    in_maps = []
    for core in range(N_CORES):
        shard = x_flat[core * ROWS_PER_CORE:(core + 1) * ROWS_PER_CORE]
        in_maps.append({
            "x": np.ascontiguousarray(shard.reshape(ROWS_PER_CORE * S, D)),
            "w1": W1, "w2": W2, "expb": expB, "ec": ec,
        })
    res = run_bass_kernel_spmd(nc, in_maps, core_ids=list(range(N_CORES)))
    out = np.empty((B * HW, S, D), np.float32)
    for core in range(N_CORES):
        out[core * ROWS_PER_CORE:(core + 1) * ROWS_PER_CORE] = (
            res.results[core]["out"].reshape(ROWS_PER_CORE, S, D))
    return out.reshape(B, HW, S, D)


# revision 13
# speedup vs baseline: 2.3419x; 1.2961x over previous
"""Trainium2 Bass kernel for LocalSelfAttention (sliding-window attention).

Reference computation (fp32):
  qkv = x @ W_qkv ; q /= 8 ; sliding window of 7 keys (3 each side, zero-padded)
  attn = softmax(q . k_win + pos_bias) ; out = (attn @ v_win) @ W_out

Sharding: data-parallel over B*HW = 128 independent rows -> 16 rows per core.
Each core processes its rows in 8 pairs (512 tokens per pair).

Per-core layout, software-pipelined 4 pairs deep so the PE never stalls on
the vector-engine softmax chain:
  stage A(p):  xT arrives via DMA-transpose (x pre-cast to bf16 on host);
               qkT = W_qk^T. @ xT in fp8 DoubleRow (2 rows/cycle);
               V = xT^T. @ W_v (bf16); scores ST[key,q] per head;
               exp (Scalar); * expB band mask (Vector)
  stage B1(p): denom = ones^T. @ attn_un (replicated across 64 partitions,
               2 heads per psum tile), + edge-correction,
               reciprocal_approx_fast (Vector)
  stage B2(p): avT = V^T. @ attn_un; * recip while copying PSUM->SBUF
  stage C(p):  out = avT^T. @ W_out, DMA'd straight from PSUM

Emission order per step t: A(t), B1(t-1), B2(t-2), C(t-3).
"""

import numpy as np
import ml_dtypes

import concourse.bass as bass
import concourse.tile as tile
from concourse import bacc, mybir
from concourse.bass_utils import run_bass_kernel_spmd

# Problem constants (hardcoded per contract)
B, HW, S, D = 2, 64, 256, 512
HEADS, DK, KSIZE, PAD = 8, 64, 7, 3
HDK = HEADS * DK            # 512
QK = 2 * HDK                # 1024 (q and k dims)
N_CORES = 8
ROWS_PER_CORE = (B * HW) // N_CORES   # 16
PAIRS = ROWS_PER_CORE // 2            # 8
PTOK = 2 * S                          # 512 tokens per pair
P = 128
NCH = S // P                          # 2 key chunks per row
STRIPE = 132                          # query stripe width per key chunk (even)
STRIPE_PAD = 256                      # psum slot per (chunk,row) stripe, bank aligned
STRIPE_START = (0, S - STRIPE)        # stripe start per chunk within a row
HPAIRS = HEADS // 2                   # 4 head pairs packed on 64+64 partitions

F32 = mybir.dt.float32
BF16 = mybir.dt.bfloat16
FP8 = mybir.dt.float8e4
FP8_WSCALE = 32.0                     # fp8 qk weight pre-scale (power of 2)
EXP_SCALE = 1.0 / (FP8_WSCALE * FP8_WSCALE * 8.0)  # undo w-scales + 1/sqrt(DK)

_CACHE = {}


def _host_constants(pos_bias, W_qkv, W_out):
    """Host-precomputed tensors: fp8 qk weights (q pre-scaled), bf16 v/out
    weights, expB band mask, head-pair-packed edge correction."""
    W1 = W_qkv.astype(np.float32).copy()
    # qk weights scaled up by 32 to sit in fp8e4m3's normal range (std 0.02
    # would straddle the 2^-6 min normal); compensated in the exp scale,
    # which also folds in the 1/sqrt(DK) query scaling.
    W1qk = (W1[:, :QK] * FP8_WSCALE).astype(ml_dtypes.float8_e4m3)  # [512, 1024]
    W1v = W1[:, QK:].astype(ml_dtypes.bfloat16)               # [512, 512]
    W2 = W_out.astype(np.float32).astype(ml_dtypes.bfloat16)  # [512, 512]

    pb = pos_bias.astype(np.float32)              # [H, S, KSIZE]
    # expB[j, h, c, q'] : key j (within chunk c), query q = STRIPE_START[c] + q'
    # value exp(pos_bias[h, q, w]) with w = (j_global - q) + PAD if in band else 0
    j = np.arange(P)[:, None, None, None]
    h = np.arange(HEADS)[None, :, None, None]
    c = np.arange(NCH)[None, None, :, None]
    qp = np.arange(STRIPE)[None, None, None, :]
    q_glob = np.array(STRIPE_START)[None, None, :, None] + qp
    j_glob = c * P + j
    w = j_glob - q_glob + PAD
    in_band = (w >= 0) & (w < KSIZE)
    w_c = np.clip(w, 0, KSIZE - 1)
    bias_val = pb[h, q_glob, w_c]
    expB = np.where(in_band, np.exp(bias_val), 0.0).astype(np.float32)
    expB = expB.astype(ml_dtypes.bfloat16)        # [128, H, NCH, STRIPE]

    # edge correction: sum over out-of-range window slots of exp(bias).
    # Packed per head pair: partitions 0:64 <- head 2j, 64:128 <- head 2j+1,
    # matching the tile_position packing of the denominator matmuls.
    q = np.arange(S)[None, :, None]
    w2 = np.arange(KSIZE)[None, None, :]
    oor = ((q + w2 - PAD) < 0) | ((q + w2 - PAD) >= S)
    ec = (np.exp(pb) * oor).sum(-1)               # [H, S]
    ec_pair = np.concatenate([ec, ec], axis=1)    # [H, PTOK]
    ec_pack = np.empty((P, HPAIRS, PTOK), np.float32)
    for jj in range(HPAIRS):
        ec_pack[:64, jj, :] = ec_pair[2 * jj][None, :]
        ec_pack[64:, jj, :] = ec_pair[2 * jj + 1][None, :]
    return W1qk, W1v, W2, expB, ec_pack


def _build_nc():
    nc = bacc.Bacc(None, target_bir_lowering=False)
    x_d = nc.dram_tensor("x", [ROWS_PER_CORE * S, D], BF16, kind="ExternalInput")
    w1qk_d = nc.dram_tensor("w1qk", [D, QK], FP8, kind="ExternalInput")
    w1v_d = nc.dram_tensor("w1v", [D, HDK], BF16, kind="ExternalInput")
    w2_d = nc.dram_tensor("w2", [HDK, D], BF16, kind="ExternalInput")
    expb_d = nc.dram_tensor("expb", [P, HEADS, NCH, STRIPE], BF16, kind="ExternalInput")
    ec_d = nc.dram_tensor("ec", [P, HPAIRS, PTOK], F32, kind="ExternalInput")
    out_d = nc.dram_tensor("out", [ROWS_PER_CORE * S, D], F32, kind="ExternalOutput")

    KO = D // P      # 4 K-chunks for projections
    TC = PTOK // P   # 4 token chunks per pair
    QKC = QK // P    # 8 qk output chunks
    HC = HDK // P    # 4 hdk chunks
    DR = mybir.MatmulPerfMode.DoubleRow

    with tile.TileContext(nc) as tc:
        with (
            tc.tile_pool(name="const", bufs=1) as const,
            tc.tile_pool(name="io", bufs=3) as io,
            tc.tile_pool(name="early", bufs=2) as early,
            tc.tile_pool(name="vpool", bufs=3) as vpool,
            tc.tile_pool(name="attn", bufs=3) as attnp,
            tc.tile_pool(name="bpool", bufs=2) as bpool,
            tc.tile_pool(name="ps_proj", bufs=3, space="PSUM") as ps_proj,
            tc.tile_pool(name="ps_st", bufs=2, space="PSUM") as ps_st,
        ):
            # ---- constants (sync queue; x traffic rides the scalar queue) ----
            w1qk_sb = const.tile([P, KO, QK], FP8)
            nc.sync.dma_start(w1qk_sb[:], w1qk_d.rearrange("(ko ki) n -> ki ko n", ki=P))
            w1v_sb = const.tile([P, KO, HDK], BF16)
            nc.sync.dma_start(w1v_sb[:], w1v_d.rearrange("(ko ki) n -> ki ko n", ki=P))
            expb_sb = const.tile([P, HEADS, NCH, STRIPE], BF16)
            nc.sync.dma_start(expb_sb[:], expb_d[:])
            w2_sb = const.tile([P, HC, D], BF16)
            nc.sync.dma_start(w2_sb[:], w2_d.rearrange("(hc ki) n -> ki hc n", ki=P))
            ec_sb = const.tile([P, HPAIRS, PTOK], F32)
            nc.sync.dma_start(ec_sb[:], ec_d[:])
            ones_sb = const.tile([P, 64], BF16)
            nc.vector.memset(ones_sb, 1.0)

            xT_tiles = {}
            attn_tiles = {}
            recip_tiles = {}
            v_tiles = {}
            avT_tiles = {}

            def stage_load(pr):
                # DMA-transpose: x [tokens, D] bf16 -> xT[p, ko, t] = x[t, ko*128+p]
                xT = io.tile([P, KO, PTOK], BF16, tag="xT")
                nc.scalar.dma_start_transpose(
                    xT[:], x_d[pr * PTOK:(pr + 1) * PTOK, :])
                xT_tiles[pr] = xT

            def stage_a(pr):
                xT = xT_tiles.pop(pr)
                xT8 = early.tile([P, KO, PTOK], FP8, tag="xT8")
                nc.vector.tensor_copy(xT8[:], xT[:])

                # qk projection in fp8 DoubleRow: qkT [qk dims, tokens]
                qkT = early.tile([P, QKC, PTOK], BF16, tag="qkT")
                for m in range(QKC):
                    pp = ps_proj.tile([P, PTOK], F32, tag="p512")
                    for kp in range(KO // 2):
                        nc.tensor.matmul(
                            pp[:],
                            w1qk_sb[:, 2 * kp:2 * kp + 2, m * P:(m + 1) * P],
                            xT8[:, 2 * kp:2 * kp + 2, :],
                            start=(kp == 0), stop=(kp == KO // 2 - 1),
                            perf_mode=DR,
                        )
                    nc.scalar.activation(
                        qkT[:, m, :], pp[:],
                        func=mybir.ActivationFunctionType.Copy)

                # v projection (bf16): V [tokens, hdk]
                v_sb = vpool.tile([P, TC, HDK], BF16, tag="v_sb")
                v_tiles[pr] = v_sb
                for tcc in range(TC):
                    pp = ps_proj.tile([P, PTOK], F32, tag="p512")
                    for ko in range(KO):
                        nc.tensor.matmul(
                            pp[:],
                            xT[:, ko, tcc * P:(tcc + 1) * P],
                            w1v_sb[:, ko, :],
                            start=(ko == 0), stop=(ko == KO - 1),
                        )
                    nc.scalar.activation(
                        v_sb[:, tcc, :], pp[:],
                        func=mybir.ActivationFunctionType.Copy)

                # scores + exp + band mask, all 8 heads
                attn_uns = []
                for h in range(HEADS):
                    mq = h // 2          # q chunk index in qkT
                    mk = 4 + h // 2      # k chunk index in qkT
                    p0 = 64 * (h % 2)    # partition offset within chunk
                    sl = slice(p0, p0 + 64)

                    st = ps_st.tile([P, NCH, 2, STRIPE_PAD], F32, tag="st")
                    for c in range(NCH):
                        for r in range(2):
                            nc.tensor.matmul(
                                st[:, c, r, :STRIPE],
                                qkT[sl, mk,
                                    r * S + c * P:r * S + (c + 1) * P],
                                qkT[sl, mq,
                                    r * S + STRIPE_START[c]:
                                    r * S + STRIPE_START[c] + STRIPE],
                                start=True, stop=True,
                            )
                    attn_un = attnp.tile(
                        [P, NCH, 2, STRIPE], BF16, tag=f"attn_un{h}")
                    nc.scalar.activation(
                        attn_un[:], st[:, :, :, :STRIPE],
                        func=mybir.ActivationFunctionType.Exp,
                        scale=EXP_SCALE)
                    nc.vector.tensor_tensor(
                        attn_un[:], attn_un[:],
                        expb_sb[:, h, :, None, :].to_broadcast(
                            (P, NCH, 2, STRIPE)),
                        mybir.AluOpType.mult,
                    )
                    attn_uns.append(attn_un)
                attn_tiles[pr] = attn_uns

            def stage_b1(pr):
                # denominators, packed 2 heads per psum tile (64+64 partitions)
                attn_uns = attn_tiles[pr]
                recip_rep = bpool.tile([P, HPAIRS, PTOK], F32, tag="recip_rep")
                recip_tiles[pr] = recip_rep
                for j in range(HPAIRS):
                    den = ps_proj.tile([P, PTOK], F32, tag="p512")
                    for h in (2 * j, 2 * j + 1):
                        p0 = 64 * (h % 2)
                        sl = slice(p0, p0 + 64)
                        tpos = None if p0 == 0 else (0, 64)
                        first = True
                        for r in range(2):
                            for c in range(NCH):
                                nc.tensor.matmul(
                                    den[sl, r * S + STRIPE_START[c]:
                                            r * S + STRIPE_START[c] + STRIPE],
                                    ones_sb[:],
                                    attn_uns[h][:, c, r, :],
                                    start=first,
                                    stop=(r == 1 and c == NCH - 1),
                                    tile_position=tpos,
                                )
                                first = False
                    nc.vector.tensor_tensor(
                        recip_rep[:, j, :], den[:],
                        ec_sb[:, j, :], mybir.AluOpType.add)
                    nc.vector.reciprocal_approx_fast(
                        recip_rep[:, j, :], recip_rep[:, j, :])

            def stage_b2(pr):
                # avT[dk, tokens] = sum over key chunks V^T. @ attn_un,
                # normalized by recip during PSUM -> SBUF
                attn_uns = attn_tiles.pop(pr)
                recip_rep = recip_tiles.pop(pr)
                v_sb = v_tiles.pop(pr)
                avT = attnp.tile([P, HC, PTOK], BF16, tag="avT")
                avT_tiles[pr] = avT
                for j in range(HPAIRS):
                    avp = ps_proj.tile([P, PTOK], F32, tag="p512")
                    for h in (2 * j, 2 * j + 1):
                        p0 = 64 * (h % 2)
                        sl = slice(p0, p0 + 64)
                        tpos = None if p0 == 0 else (0, 64)
                        first = True
                        for r in range(2):
                            for c in range(NCH):
                                nc.tensor.matmul(
                                    avp[sl, r * S + STRIPE_START[c]:
                                            r * S + STRIPE_START[c] + STRIPE],
                                    v_sb[:, 2 * r + c, h * DK:(h + 1) * DK],
                                    attn_uns[h][:, c, r, :],
                                    start=first,
                                    stop=(r == 1 and c == NCH - 1),
                                    tile_position=tpos,
                                )
                                first = False
                    nc.vector.tensor_tensor(
                        avT[:, j, :], avp[:],
                        recip_rep[:, j, :],
                        mybir.AluOpType.mult,
                    )

            def stage_c(pr):
                avT = avT_tiles.pop(pr)
                o_sb = bpool.tile([P, TC, D], F32, tag="o_sb")
                for tcc in range(TC):
                    pp = ps_proj.tile([P, PTOK], F32, tag="p512")
                    for hc in range(HC):
                        nc.tensor.matmul(
                            pp[:],
                            avT[:, hc, tcc * P:(tcc + 1) * P],
                            w2_sb[:, hc, :],
                            start=(hc == 0), stop=(hc == HC - 1),
                        )
                    nc.vector.tensor_copy(o_sb[:, tcc, :], pp[:])
                nc.sync.dma_start(
                    out_d[pr * PTOK:(pr + 1) * PTOK, :].rearrange(
                        "(tc p) d -> p tc d", p=P),
                    o_sb[:],
                )

            # ---- software pipeline: A(t), B1(t-1), B2(t-2), C(t-3) ----
            stage_load(0)
            if PAIRS > 1:
                stage_load(1)
            for t in range(PAIRS + 3):
                if t < PAIRS:
                    stage_a(t)
                    if t + 2 < PAIRS:
                        stage_load(t + 2)
                if 0 <= t - 1 < PAIRS:
                    stage_b1(t - 1)
                if 0 <= t - 2 < PAIRS:
                    stage_b2(t - 2)
                if 0 <= t - 3 < PAIRS:
                    stage_c(t - 3)

    nc.compile()
    return nc


def _in_maps(inputs, pos_bias, W_qkv, W_out):
    x = np.asarray(inputs, np.float32).astype(ml_dtypes.bfloat16)
    W1qk, W1v, W2, expB, ec = _host_constants(
        np.asarray(pos_bias), np.asarray(W_qkv), np.asarray(W_out))
    x_flat = x.reshape(B * HW, S, D)
    in_maps = []
    for core in range(N_CORES):
        shard = x_flat[core * ROWS_PER_CORE:(core + 1) * ROWS_PER_CORE]
        in_maps.append({
            "x": np.ascontiguousarray(shard.reshape(ROWS_PER_CORE * S, D)),
            "w1qk": W1qk, "w1v": W1v, "w2": W2, "expb": expB, "ec": ec,
        })
    return in_maps


def kernel(inputs, pos_bias, W_qkv, W_out):
    if "nc" not in _CACHE:
        _CACHE["nc"] = _build_nc()
    nc = _CACHE["nc"]

    in_maps = _in_maps(inputs, pos_bias, W_qkv, W_out)
    res = run_bass_kernel_spmd(nc, in_maps, core_ids=list(range(N_CORES)))
    out = np.empty((B * HW, S, D), np.float32)
    for core in range(N_CORES):
        out[core * ROWS_PER_CORE:(core + 1) * ROWS_PER_CORE] = (
            res.results[core]["out"].reshape(ROWS_PER_CORE, S, D))
    return out.reshape(B, HW, S, D)
